# revision 6
# baseline (speedup 1.0000x reference)
"""DeepSeekMoE kernel for 8 Trainium2 NeuronCores.

Strategy: data-parallel over tokens (each core owns T/8 = 1024 tokens,
all experts replicated). Per core, everything runs on device:
  - router logits via exact-fp32 DVE dot products (top-2 selection is
    sensitive to ~1e-6 logit noise, so the PE's reduced-precision
    float32r path is not used for the router)
  - top-2 + renormalize: renormalized top-2 softmax weights equal
    sigmoid(l_e - l_other), computed token-major with nc.vector.max
  - dense per-expert SwiGLU (PE matmuls in float32r: full rate, ~1e-4
    rel err) with the per-token combine weight folded into the hidden
    activations before the down projection, so the routed-expert sum
    accumulates for free in SBUF
  - shared experts use the same pipeline with weight 1
Outputs are disjoint token slices; host just concatenates.
"""

import sys

sys.path.insert(0, "/opt/trn_rl_repo")

import numpy as np

B, L, D = 4, 2048, 1024
E, KTOP, S = 8, 2, 2
F = 1408
NCORES = 8
T = B * L                 # 8192 tokens
TL = T // NCORES          # 1024 tokens per core
P = 128
DO = D // P               # 8 d-tiles
FO = F // P               # 11 f-tiles
NTT = TL // 512           # 2 token tiles of 512
TO = TL // P              # 8 token tiles of 128
NE = S + E                # shared experts first, then routed

_CACHE = {}


def _build():
    import concourse.bass as bass
    import concourse.bacc as bacc
    import concourse.mybir as mybir
    import concourse.tile as tile
    from concourse.masks import make_identity

    F32 = mybir.dt.float32
    F32R = mybir.dt.float32r
    AF = mybir.ActivationFunctionType
    OP = mybir.AluOpType
    AX = mybir.AxisListType

    nc = bacc.Bacc("TRN2", target_bir_lowering=False, debug=False,
                   num_devices=NCORES)

    x_d = nc.dram_tensor("x", [TL, D], F32, kind="ExternalInput")
    gwT_d = nc.dram_tensor("gwT", [E, D], F32, kind="ExternalInput")
    eg_d = nc.dram_tensor("exp_gate", [E, D, F], F32R, kind="ExternalInput")
    eu_d = nc.dram_tensor("exp_up", [E, D, F], F32R, kind="ExternalInput")
    ed_d = nc.dram_tensor("exp_down", [E, F, D], F32R, kind="ExternalInput")
    sg_d = nc.dram_tensor("sh_gate", [S, D, F], F32R, kind="ExternalInput")
    su_d = nc.dram_tensor("sh_up", [S, D, F], F32R, kind="ExternalInput")
    sd_d = nc.dram_tensor("sh_down", [S, F, D], F32R, kind="ExternalInput")
    out_d = nc.dram_tensor("out", [TL, D], F32, kind="ExternalOutput")

    with tile.TileContext(nc) as tc:
        with (
            tc.tile_pool(name="big", bufs=1) as big,        # per-expert C
            tc.tile_pool(name="persist", bufs=1) as persist,
            tc.tile_pool(name="wpool", bufs=2) as wpool,    # wg/wu streaming
            tc.tile_pool(name="wdpool", bufs=1) as wdpool,  # x_sb then Wd's
            tc.tile_pool(name="wbcpool", bufs=1) as wbcpool,
            tc.tile_pool(name="scr", bufs=2) as scr,
            tc.tile_pool(name="ps", bufs=2, space="PSUM") as ps,
        ):
            # ---------- Phase A: load x (token-major), build Xt ----------
            ident = persist.tile([P, P], F32, tag="ident")
            make_identity(nc, ident[:])

            x_sb_full = wdpool.tile([P, FO, D], F32, tag="wd", name="x_sb")
            x_sb = x_sb_full[:, :TO, :]
            nc.sync.dma_start(
                x_sb[:], x_d.ap().rearrange("(to p) d -> p to d", p=P)
            )
            xt = persist.tile([P, DO, TL], F32R, tag="xt")
            for to in range(TO):
                for do in range(DO):
                    tr = ps.tile([P, 512], F32, tag="h1")
                    nc.tensor.transpose(
                        tr[:, :P], x_sb[:, to, do * P:(do + 1) * P], ident[:]
                    )
                    nc.vector.tensor_copy(
                        xt[:, do, to * P:(to + 1) * P], tr[:, :P]
                    )

            # ---------- Phase B: router (exact fp32 on DVE) ----------
            lg_tok = persist.tile([P, TO, E], F32, tag="lg")
            for e in range(E):
                gwb = wpool.tile([P, D], F32, tag="gwb")
                src = gwT_d.ap()[e:e + 1, :]
                nc.sync.dma_start(
                    gwb[:],
                    bass.AP(tensor=src.tensor, offset=src.offset,
                            ap=[[0, P], [1, D]]),
                )
                for to in range(TO):
                    junk = wpool.tile([P, D], F32, tag="junk")
                    nc.vector.tensor_mul(junk[:], x_sb[:, to, :], gwb[:])
                    part = scr.tile([P, 16], F32, tag="part")
                    nc.vector.tensor_reduce(
                        out=part[:],
                        in_=junk[:].rearrange("p (a b) -> p a b", a=16),
                        axis=AX.X, op=OP.add,
                    )
                    nc.vector.tensor_reduce(
                        out=lg_tok[:, to, e:e + 1], in_=part[:],
                        axis=AX.X, op=OP.add,
                    )

            # top-2 weights, token-major, then transpose to expert-major
            wE = persist.tile([E, TL], F32, tag="wE")
            for to in range(TO):
                lt = lg_tok[:, to, :]                        # [128, 8]
                mx = scr.tile([P, 8], F32, tag="mx")
                nc.vector.max(mx[:], lt)
                s12 = scr.tile([P, 1], F32, tag="s12")
                nc.vector.tensor_add(s12[:], mx[:, 0:1], mx[:, 1:2])
                arg = scr.tile([P, E], F32, tag="arg")
                nc.vector.tensor_scalar(
                    out=arg[:], in0=lt, scalar1=2.0, scalar2=s12[:],
                    op0=OP.mult, op1=OP.subtract,
                )
                sig = scr.tile([P, E], F32, tag="sig")
                nc.scalar.activation(sig[:], arg[:], AF.Sigmoid)
                msk = scr.tile([P, E], F32, tag="msk")
                nc.vector.tensor_scalar(
                    out=msk[:], in0=lt, scalar1=mx[:, 1:2], scalar2=None,
                    op0=OP.is_ge,
                )
                wtok = scr.tile([P, E], F32, tag="wtok")
                nc.vector.tensor_mul(wtok[:], sig[:], msk[:])
                tp = ps.tile([P, P], F32, tag="tp")
                nc.tensor.transpose(tp[:8, :], wtok[:], ident[:])
                nc.vector.tensor_copy(wE[:, to * P:(to + 1) * P], tp[:8, :])

            # ---------- Phase C: experts ----------
            acc = persist.tile([P, TO, D], F32, tag="acc")  # token-major y
            for ei in range(NE):
                shared = ei < S
                if shared:
                    wg_src = sg_d.ap()[ei]
                    wu_src = su_d.ap()[ei]
                    wd_src = sd_d.ap()[ei]
                    wbc = None
                else:
                    e = ei - S
                    wg_src = eg_d.ap()[e]
                    wu_src = eu_d.ap()[e]
                    wd_src = ed_d.ap()[e]
                    # copy expert row e to partition 0, broadcast on-chip
                    w0 = wbcpool.tile([P, TL], F32, tag="w0")
                    nc.sync.dma_start(w0[0:1, :], wE[e:e + 1, :])
                    wbc = wbcpool.tile([P, TL], F32, tag="wbc")
                    nc.gpsimd.partition_broadcast(wbc[:], w0[0:1, :])

                wd_full = wdpool.tile([P, FO, D], F32R, tag="wd")
                nc.sync.dma_start(
                    wd_full[:], wd_src.rearrange("(fo p) d -> p fo d", p=P)
                )

                C = big.tile([P, FO, TL], F32R, tag="big")
                for f in range(FO):
                    wg_t = wpool.tile([P, DO, P], F32R, tag="wg")
                    nc.sync.dma_start(
                        wg_t[:],
                        wg_src[:, f * P:(f + 1) * P].rearrange(
                            "(do p) f -> p do f", p=P),
                    )
                    wu_t = wpool.tile([P, DO, P], F32R, tag="wu")
                    nc.sync.dma_start(
                        wu_t[:],
                        wu_src[:, f * P:(f + 1) * P].rearrange(
                            "(do p) f -> p do f", p=P),
                    )
                    for tt in range(NTT):
                        tsl = slice(tt * 512, (tt + 1) * 512)
                        h1 = ps.tile([P, 512], F32, tag="h1")
                        for do in range(DO):
                            nc.tensor.matmul(
                                h1[:], wg_t[:, do, :], xt[:, do, tsl],
                                start=(do == 0), stop=(do == DO - 1),
                            )
                        h2 = ps.tile([P, 512], F32, tag="h2")
                        for do in range(DO):
                            nc.tensor.matmul(
                                h2[:], wu_t[:, do, :], xt[:, do, tsl],
                                start=(do == 0), stop=(do == DO - 1),
                            )
                        cs = C[:, f, tsl]
                        nc.scalar.activation(cs, h1[:], AF.Silu)
                        nc.vector.tensor_tensor(
                            out=cs, in0=cs.bitcast(F32), in1=h2[:], op=OP.mult
                        )
                        if not shared:
                            nc.vector.tensor_tensor(
                                out=cs, in0=cs.bitcast(F32), in1=wbc[:, tsl],
                                op=OP.mult,
                            )

                # down projection straight into token-major layout
                for to in range(TO):
                    for dh in range(D // 512):
                        dn = ps.tile([P, 512], F32, tag="dn")
                        for f in range(FO):
                            nc.tensor.matmul(
                                dn[:], C[:, f, to * P:(to + 1) * P],
                                wd_full[:, f, dh * 512:(dh + 1) * 512],
                                start=(f == 0), stop=(f == FO - 1),
                            )
                        slot = acc[:, to, dh * 512:(dh + 1) * 512]
                        if ei == 0:
                            nc.vector.tensor_copy(slot, dn[:])
                        else:
                            nc.vector.tensor_add(slot, slot, dn[:])

            # ---------- output ----------
            nc.sync.dma_start(
                out_d.ap().rearrange("(to p) d -> p to d", p=P), acc[:]
            )

    nc.compile()
    return nc


def _get_nc():
    if "nc" not in _CACHE:
        _CACHE["nc"] = _build()
    return _CACHE["nc"]


# set by test harnesses that want an NTFF profile
TRACE = False
LAST_RESULT = None


def kernel(hidden_states, gate_w, exp_gate, exp_up, exp_down,
           sh_gate, sh_up, sh_down):
    global LAST_RESULT
    from concourse import bass_utils

    x = np.ascontiguousarray(np.asarray(hidden_states, np.float32)).reshape(T, D)
    gwT = np.ascontiguousarray(np.asarray(gate_w, np.float32).T)
    eg = np.ascontiguousarray(np.asarray(exp_gate, np.float32))
    eu = np.ascontiguousarray(np.asarray(exp_up, np.float32))
    ed = np.ascontiguousarray(np.asarray(exp_down, np.float32))
    sg = np.ascontiguousarray(np.asarray(sh_gate, np.float32))
    su = np.ascontiguousarray(np.asarray(sh_up, np.float32))
    sd = np.ascontiguousarray(np.asarray(sh_down, np.float32))

    nc = _get_nc()
    in_maps = []
    for c in range(NCORES):
        in_maps.append({
            "x": x[c * TL:(c + 1) * TL],
            "gwT": gwT,
            "exp_gate": eg,
            "exp_up": eu,
            "exp_down": ed,
            "sh_gate": sg,
            "sh_up": su,
            "sh_down": sd,
        })
    res = bass_utils.run_bass_kernel_spmd(
        nc, in_maps, core_ids=list(range(NCORES)), trace=TRACE
    )
    LAST_RESULT = res
    out = np.concatenate([res.results[c]["out"] for c in range(NCORES)], axis=0)
    return out.reshape(B, L, D)
